# revision 30
# baseline (speedup 1.0000x reference)
"""Trainium2 Bass kernel for batch-axis-softmax dot-product attention.

Problem: B=8, S=4096, D=64 fp32.
    scores = einsum('bqd,bkd->bqk', Q, K) / 8
    attn   = softmax(scores, axis=0)          # over the BATCH axis!
    out    = einsum('bqk,bkd->bqd', attn, V)

The batch-axis softmax couples only the 8 batch entries of a fixed (q, k)
position, so sharding over the *query* axis (512 queries per core, K/V
replicated) keeps the softmax fully local to each core.

Per-core pipeline, per k-tile (128 keys x 512 queries, all 8 batches):
  PE : scoresT[k,q] = K_tile @ Q^T   (fp16, fp32 psum; batch pairs packed
       into partition halves -> row-tiled concurrent MMs; each pair's two
       512-wide outputs land in one 2-bank psum tile)
  ACT: e-quarter = exp(0.125 * scores_pack), all four packs of one k-tile
       written into ONE contiguous [128, 4096] fp16 e-tile
  DVE: two fused tree adds halve 8 batches -> 2 partial sums, then a
       CUSTOM DVE op (ZSUM_RECIP_ANT, registered at build time) computes
       r = 1/(u0+u1) in a single pass: BITWISE_NOT exponent-flip seed +
       one inline Newton step (~2e-3 max rel err, fp16 out). This replaces
       the baseline's ScalarE ln/exp reciprocal AND the last tree level,
       taking ScalarE out of the normalizer entirely.
  DVE: W = E * r  as ONE [128, 8x512] fp16 2x-mode tensor_tensor with r
       broadcast over the 8 batch chunks via a stride-0 access pattern
  PE : outT_b[d,q] += V_tile matmul, accumulated across all 32 k-tiles in
       persistent psum (2 batches per bank via column tiling)
Epilogue: ScalarE (idle by then) copies each psum bank -> sbuf as soon as
its last AV lands, with a per-bank DMA right behind; host reassembles.

Engine budget per k-tile (measured): ACT 4x~1150ns = 4.6us busy, DVE
1224(L1)+690(L2)+~570(recip, amortized)+2290(mult) = 4.8us, PE ~3.5us.
DVE is the critical engine and runs ~100% busy through the steady state;
span = ~7us framework preamble + ~8us DMA/pipeline fill + 32 tiles of
~4.8us + ~10us tail/finalize => ~176-178us HW exec vs the ScalarE-bound
baseline's 201.5us. Measured rel err 1.34e-3 (limit 2e-2).

Scheduling lessons (measured, not guessed): coarser DVE ops (pair/quad-
fused L1/L2/mult) cut DVE busy-work by 5-7us but LOSE 5-12us of span to
pipeline stalls — every cross-engine dependency must stay one k-tile
deep. Splitting the first kt chunk into per-pack DMAs also lost ~10us:
sub-512B partition lines halve DMA throughput and the head DMAs
serialize on one queue.
"""

import numpy as np

B = 8
S = 4096
D = 64
NCORES = 8
QBLK = S // NCORES  # 512 queries per core
KT = 128            # keys per k-tile
NKT = S // KT       # 32 k-tiles
NPAIR = B // 2      # batch pairs packed into 128 partitions

# Reciprocal groups: (start_tile, ntiles). 4-tile groups amortize DVE op
# overhead; small groups at the start fill the DVE mult pipeline early
# (its pre-trigger work is only ~40% of a tile period) and small groups at
# the end shorten the post-loop tail. (All-pairs measured ~1.5us slower.)
GROUPS = [(0, 2), (2, 2), (4, 2), (6, 2), (8, 4), (12, 4), (16, 4),
          (20, 4), (24, 4), (28, 2), (30, 1), (31, 1)]

# test.py can flip these before calling kernel()
TRACE = False
TRACE_KWARGS = {}
LAST_RESULT = None  # BassKernelResults of the most recent run (for profiling)

_cache = {}


def _register_zsum_recip():
    """Register the fused r = 1/(a+b) custom DVE op (seed + 1 Newton step).

    nc.vector.reciprocal is ~6 cycles/elem and ScalarE ln/exp costs ~1.1us
    per [128,1024]; this runs at 1 elem/cycle/lane in one DVE pass and also
    absorbs the final level of the batch-sum tree. Seed trick (from
    RECIPROCAL_APPROX_FAST): BITWISE_NOT of the fp32 bit pattern flips the
    exponent so x*~x lands in [-4.5,-4]; one Chebyshev scale + one NR pass
    gives ~2e-3 max rel error, plenty under the fp16 output rounding that
    follows. 6 ALU stages of the 8 available.
    """
    from concourse import dve_ops
    from concourse.dve_spec import AluOp, Bin, C0, C1, Spec, Src0, Src1, lower
    from concourse.dve_spec import _has_src1
    from concourse.dve_uop import DveOpSpec

    name = "ZSUM_RECIP_ANT"
    if name in dve_ops._SUB_OPCODE_FOR_NAME:
        return next(op for op in dve_ops.OPS if op.name == name)

    _z = Src0 + Src1
    _nz = Bin(AluOp.BITWISE_NOT, _z, _z)
    _y0 = _nz * C0
    body = _y0 * (C1 - _z * _y0)

    def ref(in0, in1, s0, s1, imm2):
        zz = in0.astype(np.float32) + in1.astype(np.float32)
        nz = (~zz.view(np.int32)).view(np.float32)
        y0 = nz * np.float32(s0)
        return y0 * (np.float32(s1) - zz * y0)

    spec = Spec(body=body, reference=ref)
    row = dve_ops._CUSTOM_DVE_ROW_BASE + len(dve_ops.OPS)
    assert row < 0x20
    shas = {}
    for ver in ("v3", "v4"):
        s = DveOpSpec(name=name, opcode=row, uops=lower(spec, ver=ver),
                      rd1_en=_has_src1(spec))
        shas[ver] = s.sha(ver)
    op = dve_ops.DveOp(name, spec, subdim=False, uops_sha=shas)
    dve_ops.OPS.append(op)
    dve_ops.CUSTOM_DVE_SPECS[name] = spec
    dve_ops._SUB_OPCODE_FOR_NAME[name] = row
    return op


# Chebyshev-minimax seed constants (see dve_ops.RECIP_APPROX_FAST_CONSTS).
_RECIP_C0 = -0.23549792
_RECIP_C1 = 2.0017324


def _build_nc():
    from contextlib import ExitStack

    import concourse.tile as tile
    from concourse import bacc, mybir

    zsum_recip = _register_zsum_recip()

    f16 = mybir.dt.float16
    f32 = mybir.dt.float32
    Exp = mybir.ActivationFunctionType.Exp

    # Bacc (not raw Bass): its finalize() runs the legalization passes that
    # split multi-wait sync_info into EventSemaphore instructions (TRN2 allows
    # at most one wait per regular instruction). Only Exp is used, so the
    # default table-load insertion emits a single hoisted ACT_TABLE_LOAD.
    nc = bacc.Bacc()

    # Inputs pre-arranged on host into exact SBUF layouts (fp16):
    #   qt[p, j*512 + q] = Q[2j + p//64, cblk*512 + q, p%64]
    #   kt[p, j*4096 + k] = K[2j + p//64, k, p%64]
    #   vv[p, b*2048 + n*64 + d] = V[b, n*128 + p, d]
    qt_d = nc.dram_tensor("qt", [128, NPAIR * QBLK], f16, kind="ExternalInput")
    kt_d = nc.dram_tensor("kt", [128, NPAIR * S], f16, kind="ExternalInput")
    vv_d = nc.dram_tensor("vv", [128, B * NKT * D], f16, kind="ExternalInput")
    # out[j][(b%2)*64 + d, q] = out_bqd[2j + b%2, q, d]
    out_d = nc.dram_tensor("out", [NPAIR, 128, QBLK], f32, kind="ExternalOutput")

    with tile.TileContext(nc) as tc, ExitStack() as ctx:
        in_p = ctx.enter_context(tc.tile_pool(name="inp", bufs=1))
        e_p = ctx.enter_context(tc.tile_pool(name="e", bufs=6))
        w_p = ctx.enter_context(tc.tile_pool(name="w", bufs=4))
        t_p = ctx.enter_context(tc.tile_pool(name="tree", bufs=2))
        u_p = ctx.enter_context(tc.tile_pool(name="uq", bufs=2))
        r_p = ctx.enter_context(tc.tile_pool(name="rq", bufs=2))
        st_p = ctx.enter_context(tc.tile_pool(name="stage", bufs=1))
        ps_s = ctx.enter_context(tc.tile_pool(name="ps_s", bufs=2, space="PSUM"))
        ps_o = ctx.enter_context(tc.tile_pool(name="ps_o", bufs=1, space="PSUM"))

        # kt/vv/qt are loaded into SEPARATE per-chunk tiles: the dependency
        # tracker is tile-granular, so a single big kt tile made the first
        # matmul wait for every head DMA (~10.7us). With per-chunk tiles the
        # first matmul waits only on kt chunk 0 + qt pack 0 (~3us).
        CH = NPAIR * KT  # 512 columns per k-tile chunk (for both kt and vv)
        qts = [in_p.tile([128, QBLK], f16, name=f"qt{j}") for j in range(NPAIR)]
        kts = [in_p.tile([128, CH], f16, name=f"kt{t}") for t in range(NKT)]
        vvs = [in_p.tile([128, B * D], f16, name=f"vv{t}") for t in range(NKT)]

        # Issue order: tile 0 operands first, then per-tile chunks
        # interleaved kt/vv so the loop never waits on later chunks.
        nc.sync.dma_start(out=kts[0][:], in_=kt_d[:, 0:CH])
        for j in range(NPAIR):
            nc.sync.dma_start(
                out=qts[j][:], in_=qt_d[:, j * QBLK : (j + 1) * QBLK]
            )
        nc.sync.dma_start(out=vvs[0][:], in_=vv_d[:, 0:CH])
        for t in range(1, NKT):
            nc.sync.dma_start(out=kts[t][:], in_=kt_d[:, t * CH : (t + 1) * CH])
            nc.sync.dma_start(out=vvs[t][:], in_=vv_d[:, t * CH : (t + 1) * CH])

        # Persistent output accumulators: bank j holds batches 2j (parts
        # 0:64) and 2j+1 (parts 64:128), accumulated over all 32 k-tiles.
        oacc = [
            ps_o.tile([128, QBLK], f32, tag=f"oacc{j}", name=f"oacc{j}")
            for j in range(NPAIR)
        ]

        # AV matmuls pending issue; drained between score packs so PE always
        # services the (ACT-feeding) score matmuls promptly instead of
        # running long AV bursts that starve ScalarE. Interleaving AV MMs
        # of adjacent k-tiles is safe: psum accumulate-adds commute.
        av_pending = []

        def drain_av(n):
            for _ in range(min(n, len(av_pending))):
                av_pending.pop(0)()

        # Per-tile e/w buffers with per-tile tree adds and normalize
        # multiplies: finer interleave keeps every cross-engine dependency
        # (exp->tree, mult->AV) one tile deep. Coarser (pair/quad) DVE ops
        # were tried and measured SLOWER despite less DVE busy-work — the
        # longer dependency chains stall the exp->tree->recip->mult->AV
        # pipeline more than the saved op overhead buys.
        TW = 4 * 2 * QBLK  # 4096 columns of one k-tile

        def emit_scores_exp(t):
            # scores + exp; all four packs' exps land in ONE contiguous
            # [128, 4096] fp16 e-tile so the tree adds and the normalize
            # multiply below run as few wide DVE ops.
            e = e_p.tile([128, TW], f16, tag="e", name=f"e{t}")
            for j in range(NPAIR):
                sc = ps_s.tile([128, 2 * QBLK], f32, tag="sc", name=f"sc{t}_{j}")
                for m in range(2):  # m=0 -> b=2j (rows 0:64), m=1 -> b=2j+1
                    rb = m * 64
                    nc.tensor.matmul(
                        out=sc[:, m * QBLK : (m + 1) * QBLK],
                        lhsT=kts[t][rb : rb + 64, j * KT : (j + 1) * KT],
                        rhs=qts[j][rb : rb + 64, :],
                        start=True,
                        stop=True,
                        tile_position=(rb, 0),
                    )
                # E = exp(scores / sqrt(D)); scores*0.125 in [-6, 6] so no
                # max-subtraction is needed and fp16 range is safe.
                nc.scalar.activation(
                    e[:, j * 2 * QBLK : (j + 1) * 2 * QBLK], sc[:], Exp, scale=0.125
                )
                drain_av(2)
            return e

        def emit_tree(t, e, uq):
            # First two levels of the 8-batch sum on DVE (fp16 2x mode);
            # the final level is fused into the reciprocal op.
            W2 = TW // 2
            tt = t_p.tile([128, W2], f16, tag="t", name=f"t{t}")
            nc.vector.tensor_add(tt[:], e[:, :W2], e[:, W2:])
            nc.vector.tensor_add(
                uq[:, uq_pos(t) * 2 * QBLK : (uq_pos(t) + 1) * 2 * QBLK],
                tt[:, : 2 * QBLK],
                tt[:, 2 * QBLK :],
            )

        # group bookkeeping
        tile_group = {}
        for gi, (g0, gn) in enumerate(GROUPS):
            for u in range(g0, g0 + gn):
                tile_group[u] = (gi, g0, gn)

        def uq_pos(t):
            return t - tile_group[t][1]

        def emit_recip(gi, g0, gn, uq):
            # r = 1/(u0 + u1) for all gn tiles of the group in ONE custom
            # DVE op: [128, gn, 512] strided views of the group's U buffer.
            rq = r_p.tile([128, gn * QBLK], f16, tag="rq", name=f"rq{gi}")
            uqv = uq[:, : gn * 2 * QBLK].rearrange(
                "p (g c) -> p g c", g=gn
            )
            nc.vector._custom_dve(
                zsum_recip,
                out=rq[:].rearrange("p (g q) -> p g q", g=gn),
                in0=uqv[:, :, :QBLK],
                in1=uqv[:, :, QBLK:],
                s0=_RECIP_C0,
                s1=_RECIP_C1,
            )
            return rq

        def emit_mult(t, e, rq, g):
            # W_b = E_b * r, one fp16 2x-mode op for the whole k-tile with r
            # broadcast over the 8 (pack, half) chunks via a stride-0 AP.
            w = w_p.tile([128, 8 * QBLK], f16, tag="w", name=f"w{t}")
            r = rq[:, g * QBLK : (g + 1) * QBLK]
            nc.vector.tensor_mul(
                w[:].rearrange("p (a q) -> p a q", a=8),
                e[:].rearrange("p (a q) -> p a q", a=8),
                r.rearrange("p (a q) -> p a q", a=1).to_broadcast((128, 8, QBLK)),
            )
            return w

        def emit_av(t, w):
            # outT_b[d,q] += V_b[t]^T-form matmul, queued for interleaved
            # issue (see drain_av). Reverse order so the first-issued AV's
            # wait (on the mult's DVE tick) covers the others. The LAST
            # tile's AVs go in bank order instead so the epilogue copy of
            # bank j can start while banks j+1.. are still accumulating.
            def mk(b):
                j, m = b // 2, b % 2
                rb = m * 64

                def go():
                    nc.tensor.matmul(
                        out=oacc[j][rb : rb + 64, :],
                        lhsT=vvs[t][:, b * D : (b + 1) * D],
                        rhs=w[:, b * QBLK : (b + 1) * QBLK],
                        start=(t == 0),
                        stop=(t == NKT - 1),
                        tile_position=(0, rb),
                        skip_group_check=True,
                    )

                return go

            order = range(B) if t == NKT - 1 else reversed(range(B))
            for b in order:
                av_pending.append(mk(b))

        # Software pipeline: front end per tile t = scores+exp (PE+ACT) and
        # the L1 tree add (DVE); L2 per pair. Back end per GROUP, triggered
        # one tile after the group's last tree add: the fused reciprocal,
        # then the normalize multiplies + AV matmuls for every tile of the
        # group. All back-end deps are same-engine (DVE) or >=1 tile old,
        # so no engine's in-order queue head-of-line blocks on fresh data.
        trigger = {g0 + gn: (gi, g0, gn) for gi, (g0, gn) in enumerate(GROUPS)}
        uq_of_group = {}
        e_of_tile = {}

        for t in range(NKT + 1):
            if t < NKT:
                gi, g0, gn = tile_group[t]
                if uq_pos(t) == 0:
                    uq_of_group[gi] = u_p.tile(
                        [128, gn * 2 * QBLK], f16, tag="uq", name=f"uq{gi}"
                    )
                e = emit_scores_exp(t)
                e_of_tile[t] = e
                emit_tree(t, e, uq_of_group[gi])
            if t in trigger:
                gi, g0, gn = trigger[t]
                rq = emit_recip(gi, g0, gn, uq_of_group.pop(gi))
                for u in range(g0, g0 + gn):
                    w = emit_mult(u, e_of_tile.pop(u), rq, u - g0)
                    emit_av(u, w)
        # Tail: drain everything but the last tile's 8 AVs, then alternate
        # (2 AVs of bank j) -> (copy bank j) so the psum->sbuf copies overlap
        # the remaining accumulations. Copies run on ScalarE — idle by now,
        # and it sits closer to PSUM than VectorE.
        drain_av(len(av_pending) - 8)
        st = st_p.tile([128, NPAIR * QBLK], f32, tag="st")
        for j in range(NPAIR):
            drain_av(2)
            nc.scalar.copy(
                out=st[:, j * QBLK : (j + 1) * QBLK], in_=oacc[j][:]
            )
            # Per-bank DMA right behind its copy, so transfers overlap the
            # remaining copies/accumulations instead of waiting for all 4.
            nc.sync.dma_start(
                out=out_d[j], in_=st[:, j * QBLK : (j + 1) * QBLK]
            )

    return nc


def _get_nc():
    if "nc" not in _cache:
        nc = _build_nc()
        if not nc.is_finalized():
            # Runs Bacc.compile() legalization (wait splitting, reg alloc).
            nc.finalize()
        _cache["nc"] = nc
    return _cache["nc"]


def _host_prep(queries, keys, values):
    """Cast to fp16 and pre-arrange into the SBUF layouts (see _build_nc)."""
    k16 = np.asarray(keys, dtype=np.float16)
    v16 = np.asarray(values, dtype=np.float16)
    q16 = np.asarray(queries, dtype=np.float16)

    # kt[(b%2)*64+d, t*512 + (b//2)*128 + kk] = K[b, t*128+kk, d] (k-tile major)
    kt = np.ascontiguousarray(
        k16.reshape(NPAIR, 2, NKT, KT, D)
        .transpose(1, 4, 2, 0, 3)
        .reshape(128, NKT * NPAIR * KT)
    )
    # vv[p, t*512 + b*64 + d] = V[b, t*128+p, d] (k-tile major)
    vv = np.ascontiguousarray(
        v16.reshape(B, NKT, KT, D).transpose(2, 1, 0, 3).reshape(128, NKT * B * D)
    )

    qts = []
    for c in range(NCORES):
        qc = q16[:, c * QBLK : (c + 1) * QBLK, :]  # [8, 512, 64]
        qt = np.ascontiguousarray(
            qc.transpose(0, 2, 1).reshape(NPAIR, 128, QBLK).transpose(1, 0, 2).reshape(128, NPAIR * QBLK)
        )
        qts.append(qt)
    return qts, kt, vv


def kernel(queries, keys, values):
    global LAST_RESULT
    from concourse.bass_utils import run_bass_kernel_spmd

    queries = np.asarray(queries, dtype=np.float32)
    keys = np.asarray(keys, dtype=np.float32)
    values = np.asarray(values, dtype=np.float32)

    nc = _get_nc()
    qts, kt, vv = _host_prep(queries, keys, values)
    in_maps = [{"qt": qts[c], "kt": kt, "vv": vv} for c in range(NCORES)]

    res = run_bass_kernel_spmd(
        nc,
        in_maps,
        list(range(NCORES)),
        trace=TRACE,
        **TRACE_KWARGS,
    )
    LAST_RESULT = res

    out = np.empty((B, S, D), dtype=np.float32)
    for c in range(NCORES):
        o = res.results[c]["out"]  # [4, 128, 512] = [j, (b%2)*64+d, q]
        out[:, c * QBLK : (c + 1) * QBLK, :] = (
            o.reshape(B, D, QBLK).transpose(0, 2, 1)
        )
    return out
